# revision 15
# baseline (speedup 1.0000x reference)
"""Trainium2 Bass kernel for Chebyshev (L-inf) "convolution".

Math (see reference):
  out[b,co,h,w] = max_n |weights[co,n] - x_pad[b, c(co,n), h+di(co,n), w+dj(co,n)]| + bias[co]
  where conn_idx[co,n] = c*9 + di*3 + dj and x_pad is replicate-padded by 1.

Strategy (8 NeuronCores, batch-sharded: 4 images per core), v3:
  Host prep: replicate-pad x to [B, 64, 66, 66], cast to bf16, and ship the
  flattened per-core shard as the gather source ("xg", 2.23 MB/core).  All
  per-(co,n) gather offsets (element offsets into xg) are host-computed from
  conn_idx.  No on-device pad/cast/store stage: the gathers depend only on
  an external input + the offset table and fire right after program start.

  Device, per image:
    1. ONE 4-offset indirect DMA (GPSIMD ucode, 512 descriptors): partition
       co reads four contiguous 4222-element bf16 spans of xg at
       off[n] = b*278784 + c_n*4356 + di_n*66 + dj_n.  Each shifted 64x64
       window sits at row-stride 66 inside its span.
    2. Taps 0,1 on ScalarE: t = Abs(g + (-w)) activation.
       Taps 2,3 on DVE: tensor_scalar subtract (4x bf16) + sign-clear via
       bitwise_and 0x7FFF on the u16 view (bf16 abs, 4x).
    3. DVE max tree (2x bf16); ScalarE Identity activation adds bias; the
       result is stored as bf16 (host upcasts to fp32; bf16 rounding is
       ~0.4% << the 2e-2 tolerance).

  Per-core DRAM traffic: 17.3 MB gather + 4.2 MB out (vs 32.1 MB baseline).
"""

import numpy as np

B, CIN, H, W = 32, 64, 64, 64
COUT, NCONN = 128, 4
KH, KW = 3, 3
NCORES = 8
BL = B // NCORES            # 4 images per core
PH, PW = H + 2, W + 2       # 66 x 66 replicate-padded planes
PLANE = PH * PW             # 4356
IMG = CIN * PLANE           # 278784 elements per padded image
S = H * W                   # 4096
SPAN = (H - 1) * PW + W     # 4222: span holding one shifted 64x64 window
GPAD = 64 * PW              # 4224 (= 64*66) per-tap stride in the G tile

_CACHE = {}


def _build_program():
    import concourse.bass as bass
    import concourse.bacc as bacc
    import concourse.mybir as mybir
    from concourse.tile import TileContext

    f32 = mybir.dt.float32
    bf16 = mybir.dt.bfloat16
    u16 = mybir.dt.uint16
    i32 = mybir.dt.int32
    Alu = mybir.AluOpType
    Act = mybir.ActivationFunctionType

    nc = bacc.Bacc("TRN2", target_bir_lowering=False, debug=False)

    # flattened padded bf16 images for this core's 4-image shard
    xg = nc.dram_tensor("xg", (BL * IMG, 1), bf16, kind="ExternalInput")
    w_ext = nc.dram_tensor("w", (COUT, NCONN), f32, kind="ExternalInput").ap()
    wneg_ext = nc.dram_tensor("wneg", (COUT, NCONN), f32, kind="ExternalInput").ap()
    bias_ext = nc.dram_tensor("bias", (COUT, 1), f32, kind="ExternalInput").ap()
    # per (b, n): one 8-int32 slot per partition at cols [(b*NCONN+n)*8, +8);
    # the indirect-DMA ucode reads col 0 of each 32 B slot.
    gidx_ext = nc.dram_tensor(
        "gidx", (COUT, BL * NCONN * 8), i32, kind="ExternalInput"
    ).ap()
    out_ext = [
        nc.dram_tensor(f"out{b}", (COUT, S), bf16, kind="ExternalOutput").ap()
        for b in range(BL)
    ]

    with TileContext(nc, pool_alloc_mode="queue") as tc:
        with (
            tc.tile_pool(name="const", bufs=1) as cpool,
            tc.tile_pool(name="g", bufs=BL * NCONN) as gpool,
            tc.tile_pool(name="t", bufs=4) as tpool,
            tc.tile_pool(name="m", bufs=2) as mpool,
            tc.tile_pool(name="m2", bufs=2) as m2pool,
            tc.tile_pool(name="o", bufs=1) as opool,
        ):
            # gidx gates the gathers: load it on the gpsimd queue itself so
            # the first indirect DMA needs no cross-engine wait.
            gidx_sb = cpool.tile([COUT, BL * NCONN * 8], i32)
            nc.gpsimd.dma_start(out=gidx_sb[:], in_=gidx_ext)
            w_sb = cpool.tile([COUT, NCONN], f32)
            nc.sync.dma_start(out=w_sb[:], in_=w_ext)
            wneg_sb = cpool.tile([COUT, NCONN], f32)
            nc.sync.dma_start(out=wneg_sb[:], in_=wneg_ext)
            bias_sb = cpool.tile([COUT, 1], f32)
            nc.sync.dma_start(out=bias_sb[:], in_=bias_ext)
            absmask_sb = cpool.tile([COUT, 1], u16)
            nc.vector.memset(absmask_sb[:], 0x7FFF)

            # offset view [COUT, BL*NCONN, 1] with 8-int32 slot stride
            gidx_v = gidx_sb[:].rearrange(
                "p (k e) -> p k e", k=BL * NCONN, e=8
            )

            # --- all 16 gathers up front ---
            gts = []
            for b in range(BL):
                row = []
                for n in range(NCONN):
                    k = b * NCONN + n
                    gt = gpool.tile([COUT, GPAD], bf16, tag="g")
                    nc.gpsimd.indirect_dma_start(
                        out=gt[:, 0:SPAN],
                        out_offset=None,
                        in_=xg.ap(),
                        in_offset=bass.IndirectOffsetOnAxis(
                            ap=gidx_v[:, k : k + 1, 0:1], axis=0
                        ),
                    )
                    row.append(gt)
                gts.append(row)

            # --- compute, software-pipelined by one image ---
            def tap_stage(b):
                ts = []
                for n in range(NCONN):
                    gv = gts[b][n][:].rearrange(
                        "p (h w) -> p h w", h=H, w=PW
                    )[:, :, 0:W]
                    tt = tpool.tile([COUT, S], bf16, tag="t")
                    tv = tt[:].rearrange("p (h w) -> p h w", h=H, w=W)
                    if n < 2:
                        nc.scalar.activation(
                            out=tv,
                            in_=gv,
                            func=Act.Abs,
                            bias=wneg_sb[:, n : n + 1],
                            scale=1.0,
                        )
                    else:
                        nc.vector.tensor_scalar(
                            out=tv,
                            in0=gv,
                            scalar1=w_sb[:, n : n + 1],
                            scalar2=None,
                            op0=Alu.subtract,
                        )
                        nc.vector.tensor_scalar(
                            out=tt[:].bitcast(u16),
                            in0=tt[:].bitcast(u16),
                            scalar1=absmask_sb[:, 0:1],
                            scalar2=None,
                            op0=Alu.bitwise_and,
                        )
                    ts.append(tt)
                m0 = mpool.tile([COUT, S], bf16, tag="m")
                nc.vector.tensor_tensor(
                    out=m0[:], in0=ts[0][:], in1=ts[1][:], op=Alu.max
                )
                m1 = mpool.tile([COUT, S], bf16, tag="m")
                nc.vector.tensor_tensor(
                    out=m1[:], in0=ts[2][:], in1=ts[3][:], op=Alu.max
                )
                return m0, m1

            def tail_stage(b, m0, m1):
                m2 = m2pool.tile([COUT, S], bf16, tag="m2")
                nc.vector.tensor_tensor(
                    out=m2[:], in0=m0[:], in1=m1[:], op=Alu.max
                )
                ob = opool.tile([COUT, S], bf16, tag="o")
                nc.scalar.activation(
                    out=ob[:],
                    in_=m2[:],
                    func=Act.Identity,
                    bias=bias_sb[:, 0:1],
                    scale=1.0,
                )
                nc.sync.dma_start(out=out_ext[b], in_=ob[:])

            for b in range(BL):
                m0, m1 = tap_stage(b)
                tail_stage(b, m0, m1)
    nc.compile()
    return nc


def _host_inputs(x, weights, bias, conn_idx):
    """Per-core input maps.  Host prep: replicate-pad + bf16-cast x, shard by
    batch, and derive gather element-offsets from the tiny index tensor."""
    import ml_dtypes

    bf16 = ml_dtypes.bfloat16

    x = np.asarray(x, dtype=np.float32)
    xp = np.pad(x, ((0, 0), (0, 0), (1, 1), (1, 1)), mode="edge")  # [B,64,66,66]
    xpb = xp.astype(bf16).reshape(B, IMG)

    ci = np.asarray(conn_idx).astype(np.int64)          # [COUT, NCONN]
    c = ci // (KH * KW)
    rem = ci % (KH * KW)
    di = rem // KW
    dj = rem % KW
    offs = (c * PLANE + di * PW + dj).astype(np.int32)  # [COUT, NCONN]
    gidx = np.zeros((COUT, BL * NCONN * 8), dtype=np.int32)
    for bb in range(BL):
        for n in range(NCONN):
            k = bb * NCONN + n
            gidx[:, k * 8] = bb * IMG + offs[:, n]

    w2 = np.ascontiguousarray(np.asarray(weights), dtype=np.float32)
    bias2 = np.asarray(bias).reshape(COUT, 1).astype(np.float32)
    in_maps = []
    for kcore in range(NCORES):
        in_maps.append(
            {
                "xg": np.ascontiguousarray(
                    xpb[kcore * BL : (kcore + 1) * BL].reshape(BL * IMG, 1)
                ),
                "w": w2,
                "wneg": -w2,
                "bias": bias2,
                "gidx": gidx,
            }
        )
    return in_maps


def kernel(x, weights, bias, conn_idx):
    from concourse.bass_utils import run_bass_kernel_spmd

    if "nc" not in _CACHE:
        _CACHE["nc"] = _build_program()
    nc = _CACHE["nc"]
    in_maps = _host_inputs(x, weights, bias, conn_idx)
    res = run_bass_kernel_spmd(nc, in_maps, list(range(NCORES)))
    outs = [
        np.stack(
            [
                np.asarray(res.results[k][f"out{b}"])
                .astype(np.float32)
                .reshape(COUT, H, W)
                for b in range(BL)
            ]
        )
        for k in range(NCORES)
    ]
    return np.concatenate(outs, axis=0).astype(np.float32)


if __name__ == "__main__":
    nc = _build_program()
    print("program built OK")
